# revision 1
# baseline (speedup 1.0000x reference)
"""CorrLookup Trainium2 kernel.

Reference op (RAFT-style 1-D correlation pyramid lookup): for each pixel n
(N = B*H*W = 196608) and pyramid level i (row width Wi = 256 >> i), sample
the pixel's correlation row at x = disp[n]/2^i + k for k in -4..4 with 1-D
linear interpolation and zeros padding; output (B, 36, H, W).

The integer taps k mean all 9 samples of one (pixel, level) share one
interpolation weight w = frac(d) and live in a contiguous 10-element window
starting at q = n*S + floor(d) - 4 of a zero-padded flat row array (stride
S = Wi + 9, so out-of-range taps read real zeros).

The only per-pixel dynamic-address primitive this hardware reliably supports
is the indirect DMA with ONE row offset per partition (128 rows per
instruction, row payload = the source's trailing dim).  So the host
materializes a "shingled" view of the padded rows — row r = padflat[4r:4r+16]
(16 floats at stride 4, 64-byte payloads) for all levels concatenated into
one [V,16] table — and the kernel gathers row q>>2 per pixel.  The window
then sits at sub-offset delta = q&3 inside the 16-wide shingle, and the lerp
plus delta shift fuse into a 5-tap hat interpolation at position a = delta+w:

    out[k] = sum_j relu(1 - |a - j|) * g[k + j],  j = 0..4

Sharding: data-parallel over pixels; core c takes batch b = c (B == 8 ==
n_cores), so per-core outputs concatenate on batch with no communication.
"""

import numpy as np

P = 128
B, H, W = 8, 96, 256
NLVL = 4
K = 9            # taps per level
SH = 16          # shingle row width (floats)
STRIDE = 4       # shingle stride (floats)
NTAP = 5         # hat taps: delta in [0,3] + lerp neighbor
WS = [W >> i for i in range(NLVL)]


def _spec(n_pix, ws):
    """Per level: padded row stride, padded flat length, shingle rows, base."""
    ss = [w + K for w in ws]
    ls = [4 + n_pix * s + 28 for s in ss]
    vs = [(l - SH) // STRIDE + 1 for l in ls]
    bases = np.cumsum([0] + vs[:-1]).tolist()
    return ss, ls, vs, bases


def build_bass(n_pix=B * H * W // 8, ws=WS, grp=48):
    """Single-core SPMD program.  Inputs: shin [sum(vs), 16] f32 combined
    shingle table, disp [n_pix] f32.  Output: out [len(ws)*K, n_pix] f32."""
    import concourse.bass as bass
    import concourse.bacc as bacc
    import concourse.mybir as mybir
    from concourse.tile import TileContext

    f32 = mybir.dt.float32
    i32 = mybir.dt.int32
    Alu = mybir.AluOpType
    nlvl = len(ws)
    ss, ls, vs, bases = _spec(n_pix, ws)
    v_tot = sum(vs)

    t_all = n_pix // P          # pixels per partition
    assert t_all % grp == 0
    ngrp = t_all // grp         # gather groups per level

    nc = bacc.Bacc()
    shin = nc.declare_dram_parameter("shin", [v_tot, SH], f32, isOutput=False)
    disp = nc.declare_dram_parameter("disp", [n_pix], f32, isOutput=False)
    out = nc.declare_dram_parameter("out", [nlvl * K, n_pix], f32, isOutput=True)

    with TileContext(nc) as tc:
        with (
            tc.tile_pool(name="const", bufs=1) as cpool,
            tc.tile_pool(name="small", bufs=3) as spool,
            tc.tile_pool(name="hw", bufs=2) as hpool,
            tc.tile_pool(name="gath", bufs=3) as gpool,
            tc.tile_pool(name="res", bufs=3) as rpool,
        ):
            disp_t = cpool.tile([P, t_all], f32)
            nc.sync.dma_start(out=disp_t[:], in_=disp[:].rearrange("(p t) -> p t", p=P))

            for lvl in range(nlvl):
                s_l = ss[lvl]

                # q = n*s_l + floor(d);  d = disp / 2^lvl
                iota_t = spool.tile([P, t_all], i32, tag="iota")
                nc.gpsimd.iota(iota_t[:], pattern=[[s_l, t_all]], base=0,
                               channel_multiplier=t_all * s_l)
                if lvl == 0:
                    d_t = disp_t
                else:
                    d_t = spool.tile([P, t_all], f32, tag="d")
                    nc.scalar.mul(d_t[:], disp_t[:], 1.0 / (1 << lvl))
                # rounding-mode-agnostic floor (d >= 0): fi=cvt(d);
                # neg = (d - fi) < 0; floor = fi - neg; w = d - floor
                fi_t = spool.tile([P, t_all], i32, tag="fi")
                nc.vector.tensor_copy(out=fi_t[:], in_=d_t[:])
                ff_t = spool.tile([P, t_all], f32, tag="ff")
                nc.vector.tensor_copy(out=ff_t[:], in_=fi_t[:])
                werr_t = spool.tile([P, t_all], f32, tag="werr")
                nc.vector.tensor_tensor(out=werr_t[:], in0=d_t[:], in1=ff_t[:],
                                        op=Alu.subtract)
                negi_t = spool.tile([P, t_all], i32, tag="negi")
                nc.vector.tensor_scalar(out=negi_t[:], in0=werr_t[:], scalar1=0.0,
                                        scalar2=None, op0=Alu.is_lt)
                negf_t = spool.tile([P, t_all], f32, tag="negf")
                nc.vector.tensor_copy(out=negf_t[:], in_=negi_t[:])
                w_t = spool.tile([P, t_all], f32, tag="w")
                nc.vector.tensor_tensor(out=w_t[:], in0=werr_t[:], in1=negf_t[:],
                                        op=Alu.add)
                q_t = spool.tile([P, t_all], i32, tag="q")
                nc.vector.tensor_tensor(out=q_t[:], in0=iota_t[:], in1=fi_t[:],
                                        op=Alu.add)
                nc.vector.tensor_tensor(out=q_t[:], in0=q_t[:], in1=negi_t[:],
                                        op=Alu.subtract)
                # shingle row r = (q >> 2) + base_l ; sub-offset delta = q & 3
                r_t = spool.tile([P, t_all], i32, tag="r")
                nc.vector.tensor_scalar(out=r_t[:], in0=q_t[:], scalar1=2,
                                        scalar2=None, op0=Alu.arith_shift_right)
                nc.vector.tensor_scalar(out=r_t[:], in0=r_t[:], scalar1=bases[lvl],
                                        scalar2=None, op0=Alu.add)
                di_t = spool.tile([P, t_all], i32, tag="di")
                nc.vector.tensor_scalar(out=di_t[:], in0=q_t[:], scalar1=3,
                                        scalar2=None, op0=Alu.bitwise_and)
                df_t = spool.tile([P, t_all], f32, tag="df")
                nc.vector.tensor_copy(out=df_t[:], in_=di_t[:])
                a_t = spool.tile([P, t_all], f32, tag="a")
                nc.vector.tensor_tensor(out=a_t[:], in0=df_t[:], in1=w_t[:],
                                        op=Alu.add)
                # hat weights h_j = relu(1 - |a - j|), j = 0..NTAP-1
                h_ts = []
                for j in range(NTAP):
                    # h_j = relu(1 - |a - j|) = max(0, min(a-(j-1), (j+1)-a))
                    hj = hpool.tile([P, t_all], f32, tag=f"h{j}")
                    vj = hpool.tile([P, t_all], f32, tag="hv")
                    nc.vector.tensor_scalar(out=hj[:], in0=a_t[:],
                                            scalar1=float(j - 1), scalar2=None,
                                            op0=Alu.subtract)
                    nc.vector.tensor_scalar(out=vj[:], in0=a_t[:], scalar1=-1.0,
                                            scalar2=float(j + 1), op0=Alu.mult,
                                            op1=Alu.add)
                    nc.vector.tensor_tensor(out=hj[:], in0=hj[:], in1=vj[:],
                                            op=Alu.min)
                    nc.vector.tensor_scalar(out=hj[:], in0=hj[:], scalar1=0.0,
                                            scalar2=None, op0=Alu.max)
                    h_ts.append(hj)

                for g in range(ngrp):
                    g_t = gpool.tile([P, grp, SH], f32, tag="g")
                    for c in range(grp):
                        t = g * grp + c
                        nc.gpsimd.indirect_dma_start(
                            out=g_t[:, c, :],
                            out_offset=None,
                            in_=shin[:],
                            in_offset=bass.IndirectOffsetOnAxis(
                                ap=r_t[:, t : t + 1], axis=0),
                        )
                    # res[p, k, c] = sum_j h_j * g[p, c, k + j]
                    sl = slice(g * grp, (g + 1) * grp)
                    res_t = rpool.tile([P, K, grp], f32, tag="res")
                    tmp_t = rpool.tile([P, K, grp], f32, tag="tmp")
                    for j in range(NTAP):
                        gj = g_t[:, :, j : j + K].rearrange("p c k -> p k c")
                        hb = h_ts[j][:, None, sl].to_broadcast([P, K, grp])
                        if j == 0:
                            nc.vector.tensor_tensor(out=res_t[:], in0=gj, in1=hb,
                                                    op=Alu.mult)
                        else:
                            nc.vector.tensor_tensor(out=tmp_t[:], in0=gj, in1=hb,
                                                    op=Alu.mult)
                            nc.vector.tensor_tensor(out=res_t[:], in0=res_t[:],
                                                    in1=tmp_t[:], op=Alu.add)
                    dst = (
                        out[K * lvl : K * (lvl + 1), :]
                        .rearrange("k (p t) -> p k t", p=P)[:, :, sl]
                    )
                    nc.sync.dma_start(out=dst, in_=res_t[:])

    return nc


def _prep_core(corrs_core, n_pix, ws):
    """Build the combined shingle table for one core's pixel range."""
    ss, ls, vs, bases = _spec(n_pix, ws)
    parts = []
    for i, wi in enumerate(ws):
        padded = np.zeros((n_pix, ss[i]), dtype=np.float32)
        padded[:, :wi] = corrs_core[i]
        flat = np.zeros(ls[i], dtype=np.float32)
        flat[4 : 4 + n_pix * ss[i]] = padded.reshape(-1)
        sw = np.lib.stride_tricks.sliding_window_view(flat, SH)[::STRIDE]
        assert sw.shape[0] == vs[i]
        parts.append(sw)
    return np.ascontiguousarray(np.concatenate(parts, axis=0))


_CACHE = {}


def kernel(corr0, corr1, corr2, corr3, flow):
    """Full-input entry point: shard over 8 cores, run, gather."""
    from concourse.bass_utils import run_bass_kernel_spmd

    n_cores = 8
    n_pix = B * H * W // n_cores

    if "nc" not in _CACHE:
        nc = build_bass(n_pix=n_pix, ws=WS)
        nc.finalize()
        _CACHE["nc"] = nc
    nc = _CACHE["nc"]

    corrs = [
        np.asarray(c, dtype=np.float32).reshape(B * H * W, w)
        for c, w in zip((corr0, corr1, corr2, corr3), WS)
    ]
    flow = np.asarray(flow, dtype=np.float32)
    disp_full = flow[:, 0].reshape(B * H * W)

    in_maps = []
    for c in range(n_cores):
        sl = slice(c * n_pix, (c + 1) * n_pix)
        in_maps.append({
            "shin": _prep_core([cr[sl] for cr in corrs], n_pix, WS),
            "disp": np.ascontiguousarray(disp_full[sl]),
        })

    res = run_bass_kernel_spmd(nc, in_maps, list(range(n_cores)),
                               trace=_CACHE.get("trace", False))
    _CACHE["last_res"] = res
    outs = [res.results[c]["out"].reshape(NLVL * K, H, W) for c in range(n_cores)]
    return np.stack(outs, axis=0).astype(np.float32)



# revision 7
# speedup vs baseline: 3.6441x; 3.6441x over previous
"""CorrLookup Trainium2 kernel (merged-record dma_gather design).

Reference op (RAFT-style 1-D correlation pyramid lookup): for each pixel n
(N = B*H*W = 196608) and level i (row width Wi = 256 >> i), sample the
pixel's correlation row at x = disp[n]/2^i + k, k = -4..4, with 1-D linear
interpolation and zeros padding; output (B, 36, H, W).

Key identities: with t = floor(disp), floor(disp/2^i) = t >> i, so ALL four
levels' 10-float windows are determined by u = t >> 3.  The host builds one
256-byte record per (pixel, u) holding the four level slices
[row_i[(t>>i) - delta_i ...]] at fixed offsets OFF, where the per-level
sub-offset delta_i = (t>>i) & (2^(3-i)-1) has range 8/2^i.  The lerp plus
sub-offset fuse into a hat filter: out_i[k] = sum_j relu(1-|a_i-j|) *
rec[OFF_i + k + j], a_i = disp/2^i - (8>>i)*u, with 10/6/4/2 taps.

The gather is dma_gather (the one primitive that packs thousands of
independent descriptors into a single instruction: ~1 us fixed + 0.34 ns
per descriptor, vs ~1 us PER 128-descriptor instruction for indirect
DMA).  Records are 256 B (its minimum element), indices are int16 relative
to a per-call base: block of 1024 pixels * 32 records = 32768 rows =
exactly the int16 range.  Indices are consumed wrapped (idx i at partition
i%16, col i//16, replicated per Q7 core) and written out[i%128, i//128, :],
so pixels map column-major (n = col*128 + p); host transposes accordingly.

Sharding: data-parallel over pixels; core c takes batch b = c.
"""

import numpy as np

P = 128
B, H, W = 8, 96, 256
NLVL = 4
K = 9                    # taps per level
NREC = 32                # records per pixel (u = t>>3)
REC = 128                # record slots (fp16) = 256 B
OFF = [0, 18, 32, 44]    # level slice offsets inside a record
SLC = [18, 14, 12, 10]   # level slice widths
TAPS = [10, 6, 4, 2]     # hat taps per level
WS = [W >> i for i in range(NLVL)]

USE_DMA_GATHER = True


def build_bass(n_pix=B * H * W // 8, use_dma_gather=USE_DMA_GATHER):
    """Single-core SPMD program.
    Inputs: rec [n_pix*NREC, REC] f16, disp_cm [P, n_pix/P] f32 (column-major
    pixels: n = col*128 + p), disp_w [P, n_pix/16] f32 (wrapped+replicated:
    disp_w[c, m] = disp[16*m + c%16]; only used by the dma_gather path).
    Output: outd [NLVL*K, P, n_pix/P] f16 (host maps n = col*128 + p)."""
    import concourse.bass as bass
    import concourse.bacc as bacc
    import concourse.mybir as mybir
    from concourse.tile import TileContext

    f32 = mybir.dt.float32
    f16 = mybir.dt.float16
    i32 = mybir.dt.int32
    i16 = mybir.dt.int16
    Alu = mybir.AluOpType

    tcol = n_pix // P            # 192 pixel columns
    mw = n_pix // 16             # 1536 wrapped cols
    nblk = n_pix // 1024         # 24 gather blocks (1024 pixels each)
    half = tcol // 2

    nc = bacc.Bacc()
    rec = nc.declare_dram_parameter("rec", [n_pix * NREC, REC], f16, isOutput=False)
    disp_cm = nc.declare_dram_parameter("disp_cm", [P, tcol], f32, isOutput=False)
    disp_w = nc.declare_dram_parameter("disp_w", [P, mw], f32, isOutput=False)
    outd = nc.declare_dram_parameter("outd", [NLVL * K, P, tcol], f16, isOutput=True)

    def robust_floor(pool, d_t, cols, tagp):
        """floor for d >= 0, any f32->i32 rounding mode. Returns (i32, f32)."""
        fi = pool.tile([P, cols], i32, tag=tagp + "fi")
        nc.vector.tensor_copy(out=fi[:], in_=d_t[:])
        ff = pool.tile([P, cols], f32, tag=tagp + "ff")
        nc.vector.tensor_copy(out=ff[:], in_=fi[:])
        er = pool.tile([P, cols], f32, tag=tagp + "er")
        nc.vector.tensor_tensor(out=er[:], in0=d_t[:], in1=ff[:], op=Alu.subtract)
        ng = pool.tile([P, cols], i32, tag=tagp + "ng")
        nc.vector.tensor_scalar(out=ng[:], in0=er[:], scalar1=0.0, scalar2=None,
                                op0=Alu.is_lt)
        nc.vector.tensor_tensor(out=fi[:], in0=fi[:], in1=ng[:], op=Alu.subtract)
        return fi

    with TileContext(nc) as tc:
        with (
            tc.tile_pool(name="keep", bufs=1) as kp,
            tc.tile_pool(name="work", bufs=2) as wp,
            tc.tile_pool(name="res", bufs=2) as rp,
        ):
            g_full = kp.tile([P, tcol, REC], f16)

            if use_dma_gather:
                # ---- wrapped int16 record indices -------------------------
                # r_rel = 512*(m%64) + 32*(c%16) + min(floor(disp_w/8), 31)
                dw = kp.tile([P, mw], f32)
                nc.sync.dma_start(out=dw[:], in_=disp_w[:])
                iom = wp.tile([P, mw], i32, tag="iom")
                nc.gpsimd.iota(iom[:], pattern=[[0, nblk], [512, 64]], base=0,
                               channel_multiplier=0)
                ioc = wp.tile([P, 1], i32, tag="ioc")
                nc.gpsimd.iota(ioc[:], pattern=[[0, 1]], base=0,
                               channel_multiplier=32)
                nc.vector.tensor_scalar(out=ioc[:], in0=ioc[:], scalar1=511,
                                        scalar2=None, op0=Alu.bitwise_and)
                d8 = wp.tile([P, mw], f32, tag="d8")
                nc.scalar.mul(d8[:], dw[:], 0.125)
                uw = robust_floor(wp, d8, mw, "w")
                nc.vector.tensor_scalar(out=uw[:], in0=uw[:], scalar1=31,
                                        scalar2=None, op0=Alu.min)
                nc.vector.tensor_tensor(out=uw[:], in0=uw[:], in1=iom[:], op=Alu.add)
                nc.vector.tensor_tensor(out=uw[:], in0=uw[:],
                                        in1=ioc[:, 0:1].to_broadcast([P, mw]),
                                        op=Alu.add)
                r16 = kp.tile([P, mw], i16)
                nc.vector.tensor_copy(out=r16[:], in_=uw[:])

                # ---- gathers: 24 x 1024 records --------------------------
                for g in range(nblk):
                    nc.gpsimd.dma_gather(
                        out_ap=g_full[:, 8 * g : 8 * (g + 1), :],
                        in_ap=rec[32768 * g : 32768 * (g + 1), :],
                        idxs_ap=r16[:, 64 * g : 64 * (g + 1)],
                        num_idxs=1024, num_idxs_reg=1024, elem_size=REC,
                    )

            # ---- per-pixel params (pixel layout) -------------------------
            disp_t = kp.tile([P, tcol], f32)
            nc.sync.dma_start(out=disp_t[:], in_=disp_cm[:])

            if not use_dma_gather:
                # fallback: one indirect DMA per pixel column
                d8p = wp.tile([P, tcol], f32, tag="d8p")
                nc.scalar.mul(d8p[:], disp_t[:], 0.125)
                up = robust_floor(wp, d8p, tcol, "p")
                nc.vector.tensor_scalar(out=up[:], in0=up[:], scalar1=31,
                                        scalar2=None, op0=Alu.min)
                iop = wp.tile([P, tcol], i32, tag="iop")
                nc.gpsimd.iota(iop[:], pattern=[[NREC * P, tcol]], base=0,
                               channel_multiplier=NREC)
                nc.vector.tensor_tensor(out=up[:], in0=up[:], in1=iop[:], op=Alu.add)
                for t in range(tcol):
                    nc.gpsimd.indirect_dma_start(
                        out=g_full[:, t, :],
                        out_offset=None,
                        in_=rec[:],
                        in_offset=bass.IndirectOffsetOnAxis(ap=up[:, t : t + 1],
                                                            axis=0),
                    )

            d8c = wp.tile([P, tcol], f32, tag="d8c")
            nc.scalar.mul(d8c[:], disp_t[:], 0.125)
            u_t = robust_floor(wp, d8c, tcol, "c")
            nc.vector.tensor_scalar(out=u_t[:], in0=u_t[:], scalar1=31,
                                    scalar2=None, op0=Alu.min)
            u_f = kp.tile([P, tcol], f32)
            nc.vector.tensor_copy(out=u_f[:], in_=u_t[:])

            # hat weights per level: h_j = relu(1 - |a_L - j|), fp16
            h_ts = []
            for lvl in range(NLVL):
                taps = TAPS[lvl]
                a_t = wp.tile([P, tcol], f32, tag="a")
                # a = disp*2^-lvl - u_f*(8>>lvl)
                us = wp.tile([P, tcol], f32, tag="us")
                nc.vector.tensor_scalar(out=us[:], in0=u_f[:],
                                        scalar1=float(8 >> lvl), scalar2=None,
                                        op0=Alu.mult)
                dl = wp.tile([P, tcol], f32, tag="dl")
                nc.vector.tensor_scalar(out=dl[:], in0=disp_t[:],
                                        scalar1=1.0 / (1 << lvl), scalar2=None,
                                        op0=Alu.mult)
                nc.vector.tensor_tensor(out=a_t[:], in0=dl[:], in1=us[:],
                                        op=Alu.subtract)
                a16 = wp.tile([P, tcol], f16, tag="a16")
                nc.vector.tensor_copy(out=a16[:], in_=a_t[:])

                ji = wp.tile([P, taps], i32, tag="ji")
                nc.gpsimd.iota(ji[:], pattern=[[1, taps]], base=0,
                               channel_multiplier=0)
                jf = wp.tile([P, taps], f16, tag="jf")
                nc.vector.tensor_copy(out=jf[:], in_=ji[:])

                h_t = kp.tile([P, taps, tcol], f16, tag=f"h{lvl}")
                nc.vector.tensor_tensor(
                    out=h_t[:],
                    in0=a16[:, None, :].to_broadcast([P, taps, tcol]),
                    in1=jf[:, :, None].to_broadcast([P, taps, tcol]),
                    op=Alu.subtract)
                # h = max(0, min(1 + amj, 1 - amj)) = relu(1 - |a - j|)
                hv = wp.tile([P, taps, tcol], f16, tag="hv")
                nc.vector.tensor_scalar(out=hv[:], in0=h_t[:], scalar1=-1.0,
                                        scalar2=1.0, op0=Alu.mult, op1=Alu.add)
                nc.vector.tensor_scalar(out=h_t[:], in0=h_t[:], scalar1=1.0,
                                        scalar2=None, op0=Alu.add)
                nc.vector.tensor_tensor(out=h_t[:], in0=h_t[:], in1=hv[:],
                                        op=Alu.min)
                nc.vector.tensor_scalar(out=h_t[:], in0=h_t[:], scalar1=0.0,
                                        scalar2=None, op0=Alu.max)
                h_ts.append(h_t)

            # ---- interp: res[p,k,c] = sum_j h_j * rec[OFF+k+j], per half --
            for hf in range(2):
                sl = slice(hf * half, (hf + 1) * half)
                for lvl in range(NLVL):
                    taps, off = TAPS[lvl], OFF[lvl]
                    res_t = rp.tile([P, K, half], f16, tag="res")
                    tmp_t = rp.tile([P, K, half], f16, tag="tmp")
                    for j in range(taps):
                        gj = (g_full[:, sl, off + j : off + j + K]
                              .rearrange("p c k -> p k c"))
                        hb = h_ts[lvl][:, j, None, sl].to_broadcast([P, K, half])
                        if j == 0:
                            nc.vector.tensor_tensor(out=res_t[:], in0=gj, in1=hb,
                                                    op=Alu.mult)
                        else:
                            nc.vector.tensor_tensor(out=tmp_t[:], in0=gj, in1=hb,
                                                    op=Alu.mult)
                            nc.vector.tensor_tensor(out=res_t[:], in0=res_t[:],
                                                    in1=tmp_t[:], op=Alu.add)
                    dst = (outd[K * lvl : K * (lvl + 1), :, :]
                           .rearrange("k p t -> p k t")[:, :, sl])
                    nc.sync.dma_start(out=dst, in_=res_t[:])

    return nc


def _prep_core(corrs_core, n_pix):
    """Merged-record table [n_pix*NREC, REC] f16 for one core."""
    from numpy.lib.stride_tricks import sliding_window_view as swv

    recs = np.zeros((n_pix, NREC, REC), dtype=np.float16)
    strides = [8, 4, 2, 1]
    for i in range(NLVL):
        wi = WS[i]
        padded = np.zeros((n_pix, 4 + wi + 10), dtype=np.float32)
        padded[:, 4 : 4 + wi] = corrs_core[i]
        win = swv(padded, SLC[i], axis=1)[:, :: strides[i]][:, :NREC]
        recs[:, :, OFF[i] : OFF[i] + SLC[i]] = win
    return recs.reshape(n_pix * NREC, REC)


_CACHE = {}


def kernel(corr0, corr1, corr2, corr3, flow):
    """Full-input entry point: shard over 8 cores, run, gather."""
    from concourse.bass_utils import run_bass_kernel_spmd

    n_cores = 8
    n_pix = B * H * W // n_cores
    tcol = n_pix // P

    if "nc" not in _CACHE:
        nc = build_bass(n_pix=n_pix)
        nc.finalize()
        _CACHE["nc"] = nc
    nc = _CACHE["nc"]

    corrs = [
        np.asarray(c, dtype=np.float32).reshape(B * H * W, w)
        for c, w in zip((corr0, corr1, corr2, corr3), WS)
    ]
    flow = np.asarray(flow, dtype=np.float32)
    disp_full = flow[:, 0].reshape(B * H * W)

    in_maps = []
    for c in range(n_cores):
        sl = slice(c * n_pix, (c + 1) * n_pix)
        disp = np.ascontiguousarray(disp_full[sl])
        dw16 = disp.reshape(n_pix // 16, 16).T          # (16, mw)
        in_maps.append({
            "rec": _prep_core([cr[sl] for cr in corrs], n_pix),
            "disp_cm": np.ascontiguousarray(disp.reshape(tcol, P).T),
            "disp_w": np.ascontiguousarray(np.tile(dw16, (8, 1))),
        })

    res = run_bass_kernel_spmd(nc, in_maps, list(range(n_cores)),
                               trace=_CACHE.get("trace", False))
    _CACHE["last_res"] = res
    outs = []
    for c in range(n_cores):
        od = res.results[c]["outd"].reshape(NLVL * K, P, tcol)
        outs.append(np.transpose(od, (0, 2, 1)).reshape(NLVL * K, H, W))
    return np.stack(outs, axis=0).astype(np.float32)
